# revision 6
# baseline (speedup 1.0000x reference)
"""AttentionGate kernel for Trainium2 (8 NeuronCores, pure data parallel).

Reference computation (per pixel p, channels c):
    t[p] = sum_c input_[p,c]*wt[c] + bt
    g[p] = sum_c gating [p,c]*wg[c] + bg
    x[p] = sigmoid(w2 * relu(t[p]+g[p]) + b2)
    out[p,c] = input_[p,c] * x[p]

Layout strategy: keep the natural [pixel, channel] layout (channel innermost,
contiguous in HBM).  Per 128-pixel tile we load input and gating channels into
one [128, 512] SBUF tile (concatenated along free dim); one fused custom-DVE
TENSOR_TENSOR_REDUCE computes the elementwise product with the concatenated
weights AND its free-dim sum (the dot product) in a single instruction.
ScalarE does relu + sigmoid on the [128,1] result; DVE tensor_scalar
broadcasts the per-pixel gate over the 256 input channels.

Sharding: batch dim 16 -> 2 batches per core, weights replicated.
"""

import sys

import numpy as np

for _p in ("/opt/trn_rl_repo", "/opt/trn_rl_repo/concourse"):
    if _p not in sys.path:
        sys.path.append(_p)

B, H, W, C = 16, 128, 128, 256
NCORES = 8
ROWS = (B // NCORES) * H * W          # pixels per core = 32768
P = 128                                # partitions / pixels per tile
CAT = 2 * C                            # input || gating channels
TB = 8                                 # 128-pixel tiles per DMA block
NBLK = ROWS // (P * TB)                # 32 blocks

_PATCHED = False


def _apply_compat_patches():
    """Work around two ISA-encoding gaps in this container's neuronxcc walrus:

    1. EVENT_SEMAPHORE_RANGE_CLEAR (emitted by the TileContext teardown's
       sem_clear) fails codegen with "ISA wrong length".  Re-execution is
       safe without it (verified on HW), so skip the clear.
    2. The teardown drain carries one sem-wait per logical processor; this
       walrus rejects >1 sync-wait command on a NO_STRUCT ctrl instruction
       ("Too many sync wait commands").  Split the final clock wait into one
       NOP per processor instead.
    """
    global _PATCHED
    if _PATCHED:
        return
    _PATCHED = True

    import concourse.bass as bass
    import concourse.tile as tile_mod
    from bass_rust import ScopedClock, VectorClock
    from concourse.bass import SemaphoreHandle, compact_to_ranges

    def patched_clear(self, sems):
        if not sems:
            return
        sem_nums = [s.num if isinstance(s, SemaphoreHandle) else s for s in sems]
        for sem_range in compact_to_ranges(sem_nums):
            assert self._state.free_isdisjoint(sem_range)
            self.gpsimd.dma_reset(sem_range)
        self._state.prepend_free_semaphores(sem_nums)
        for poison_set in self._tile_sem_poison_stack:
            poison_set.update(sem_nums)

    bass.Bass.clear_and_free_semaphores = patched_clear

    def patched_drain_and_barrier(self, tick_clock, wait_clock):
        gc = tick_clock.global_clock
        for p in range(len(gc)):
            if gc[p] <= 0:
                continue
            vc = VectorClock()
            vc.require_at_least(p, gc[p])
            di = self.nc.sync.nop(nofuse=True)
            wait_clock.add_sem_waits(di.ins, ScopedClock({None: vc}))
        self.nc.all_engine_barrier()
        assert self.sems is not None
        popped = self.nc._tile_sem_poison_stack.pop()
        assert popped is self._sem_poison
        self.nc.clear_and_free_semaphores(list(self.sems.allocated().values()))
        self.nc.all_engine_barrier()

    tile_mod.TileContext._drain_and_barrier = patched_drain_and_barrier


def _split_multi_waits(nc):
    """This walrus build only encodes ONE sync-wait command per TPB
    instruction.  Hoist all-but-the-last wait of any instruction onto
    freshly inserted same-engine NoOps placed directly before it."""
    import concourse.mybir as mybir

    for f in nc.m.functions:
        for bb in f.blocks:
            insts = bb.instructions  # live list
            i = 0
            while i < len(insts):
                inst = insts[i]
                si = getattr(inst, "sync_info", None)
                if si is not None and len(si.on_wait) > 1:
                    extra, last = list(si.on_wait[:-1]), si.on_wait[-1]
                    for w in extra:
                        nop = mybir.InstNoOp(
                            name=nc.get_next_instruction_name(),
                            engine=inst.engine,
                            sync_info=mybir.SyncInfo(on_wait=[w], on_update=[]),
                            bass_nofuse=True,
                        )
                        insts.insert(i, nop)
                        i += 1
                    inst.sync_info = mybir.SyncInfo(
                        on_wait=[last], on_update=list(si.on_update)
                    )
                i += 1


def _build_program(bt, bg, w2, b2):
    import concourse.bass as bass
    import concourse.mybir as mybir
    from concourse.tile import TileContext

    nc = bass.Bass()
    x_d = nc.declare_dram_parameter("x", [ROWS, C], mybir.dt.float32, isOutput=False)
    g_d = nc.declare_dram_parameter("g", [ROWS, C], mybir.dt.float32, isOutput=False)
    w_d = nc.declare_dram_parameter("wcat", [P, CAT], mybir.dt.float32, isOutput=False)
    o_d = nc.declare_dram_parameter("out", [ROWS, C], mybir.dt.float32, isOutput=True)

    x_r = x_d[:].rearrange("(n t p) c -> n p t c", t=TB, p=P)
    g_r = g_d[:].rearrange("(n t p) c -> n p t c", t=TB, p=P)
    o_r = o_d[:].rearrange("(n t p) c -> n p t c", t=TB, p=P)

    f32 = mybir.dt.float32
    with TileContext(nc) as tc:
        with (
            tc.tile_pool(name="wp", bufs=1) as wp,
            tc.tile_pool(name="io", bufs=3) as io,
            tc.tile_pool(name="op", bufs=3) as op,
            tc.tile_pool(name="sc", bufs=2) as sc,
            tc.tile_pool(name="sm", bufs=8) as sm,
        ):
            wcat = wp.tile([P, CAT], f32)
            nc.sync.dma_start(wcat[:], w_d[:])
            b2t = wp.tile([P, 1], f32)
            nc.vector.memset(b2t[:], float(b2))
            bias_t = wp.tile([P, 1], f32)
            nc.vector.memset(bias_t[:], float(bt + bg))

            for n in range(NBLK):
                xg = io.tile([P, TB, CAT], f32)
                nc.sync.dma_start(xg[:, :, 0:C], x_r[n])
                nc.sync.dma_start(xg[:, :, C:CAT], g_r[n])
                ob = op.tile([P, TB, C], f32)
                for t in range(TB):
                    dump = sc.tile([P, CAT], f32)
                    s = sm.tile([P, 1], f32)
                    # dump = xg*wcat, s = per-pixel sum(dump) = t+g dot product
                    nc.vector.scalar_tensor_tensor(
                        out=dump[:],
                        in0=xg[:, t, :],
                        scalar=0.0,
                        in1=wcat[:],
                        op0=mybir.AluOpType.bypass,
                        op1=mybir.AluOpType.mult,
                        accum_out=s[:],
                    )
                    xs = sm.tile([P, 1], f32)
                    nc.scalar.activation(
                        xs[:], s[:], mybir.ActivationFunctionType.Relu,
                        bias=bias_t[:],
                    )
                    xsig = sm.tile([P, 1], f32)
                    nc.scalar.activation(
                        xsig[:], xs[:], mybir.ActivationFunctionType.Sigmoid,
                        bias=b2t[:], scale=float(w2),
                    )
                    nc.vector.tensor_scalar_mul(
                        ob[:, t, :], xg[:, t, 0:C], xsig[:]
                    )
                nc.sync.dma_start(o_r[n], ob[:])
    _split_multi_waits(nc)
    return nc


def kernel(**inputs):
    _apply_compat_patches()
    from concourse.bass_utils import run_bass_kernel_spmd

    x = np.ascontiguousarray(inputs["input_"], dtype=np.float32)
    g = np.ascontiguousarray(inputs["gating_signal"], dtype=np.float32)
    wt = np.asarray(inputs["wt"], dtype=np.float32)
    wg = np.asarray(inputs["wg"], dtype=np.float32)
    bt = float(np.asarray(inputs["bt"]))
    bg = float(np.asarray(inputs["bg"]))
    w2 = float(np.asarray(inputs["w2"]))
    b2 = float(np.asarray(inputs["b2"]))

    nc = _build_program(bt, bg, w2, b2)

    wcat = np.ascontiguousarray(
        np.tile(np.concatenate([wt, wg])[None, :], (P, 1)).astype(np.float32)
    )
    xs = x.reshape(NCORES, ROWS, C)
    gs = g.reshape(NCORES, ROWS, C)
    in_maps = [{"x": xs[i], "g": gs[i], "wcat": wcat} for i in range(NCORES)]
    res = run_bass_kernel_spmd(nc, in_maps, list(range(NCORES)))
    out = np.stack([res.results[i]["out"] for i in range(NCORES)], axis=0)
    return out.reshape(B, H, W, C)


# revision 8
# speedup vs baseline: 1.0169x; 1.0169x over previous
"""AttentionGate kernel for Trainium2 (8 NeuronCores, pure data parallel).

Reference computation (per pixel p, channels c):
    t[p] = sum_c input_[p,c]*wt[c] + bt
    g[p] = sum_c gating [p,c]*wg[c] + bg
    x[p] = sigmoid(w2 * relu(t[p]+g[p]) + b2)
    out[p,c] = input_[p,c] * x[p]

Layout strategy: keep the natural [pixel, channel] layout (channel innermost,
contiguous in HBM).  Per 128-pixel tile we load input and gating channels into
one [128, 512] SBUF tile (concatenated along free dim); one fused custom-DVE
TENSOR_TENSOR_REDUCE computes the elementwise product with the concatenated
weights AND its free-dim sum (the dot product) in a single instruction.
ScalarE does relu + sigmoid on the [128,1] result; DVE tensor_scalar
broadcasts the per-pixel gate over the 256 input channels.

Sharding: batch dim 16 -> 2 batches per core, weights replicated.
"""

import sys

import numpy as np

for _p in ("/opt/trn_rl_repo", "/opt/trn_rl_repo/concourse"):
    if _p not in sys.path:
        sys.path.append(_p)

B, H, W, C = 16, 128, 128, 256
NCORES = 8
ROWS = (B // NCORES) * H * W          # pixels per core = 32768
P = 128                                # partitions
CAT = 2 * C                            # input || gating channels
RPP = ROWS // P                        # pixel rows owned per partition = 256
TB = 16                                # rows (slots) per block
NBLK = RPP // TB                       # 16 blocks

_PATCHED = False


def _apply_compat_patches():
    """Work around two ISA-encoding gaps in this container's neuronxcc walrus:

    1. EVENT_SEMAPHORE_RANGE_CLEAR (emitted by the TileContext teardown's
       sem_clear) fails codegen with "ISA wrong length".  Re-execution is
       safe without it (verified on HW), so skip the clear.
    2. The teardown drain carries one sem-wait per logical processor; this
       walrus rejects >1 sync-wait command on a NO_STRUCT ctrl instruction
       ("Too many sync wait commands").  Split the final clock wait into one
       NOP per processor instead.
    """
    global _PATCHED
    if _PATCHED:
        return
    _PATCHED = True

    import concourse.bass as bass
    import concourse.tile as tile_mod
    from bass_rust import ScopedClock, VectorClock
    from concourse.bass import SemaphoreHandle, compact_to_ranges

    def patched_clear(self, sems):
        if not sems:
            return
        sem_nums = [s.num if isinstance(s, SemaphoreHandle) else s for s in sems]
        for sem_range in compact_to_ranges(sem_nums):
            assert self._state.free_isdisjoint(sem_range)
            self.gpsimd.dma_reset(sem_range)
        self._state.prepend_free_semaphores(sem_nums)
        for poison_set in self._tile_sem_poison_stack:
            poison_set.update(sem_nums)

    bass.Bass.clear_and_free_semaphores = patched_clear

    def patched_drain_and_barrier(self, tick_clock, wait_clock):
        gc = tick_clock.global_clock
        for p in range(len(gc)):
            if gc[p] <= 0:
                continue
            vc = VectorClock()
            vc.require_at_least(p, gc[p])
            di = self.nc.sync.nop(nofuse=True)
            wait_clock.add_sem_waits(di.ins, ScopedClock({None: vc}))
        self.nc.all_engine_barrier()
        assert self.sems is not None
        popped = self.nc._tile_sem_poison_stack.pop()
        assert popped is self._sem_poison
        self.nc.clear_and_free_semaphores(list(self.sems.allocated().values()))
        self.nc.all_engine_barrier()

    tile_mod.TileContext._drain_and_barrier = patched_drain_and_barrier


def _split_multi_waits(nc):
    """This walrus build only encodes ONE sync-wait command per TPB
    instruction.  Hoist all-but-the-last wait of any instruction onto
    freshly inserted same-engine NoOps placed directly before it."""
    import concourse.mybir as mybir

    for f in nc.m.functions:
        for bb in f.blocks:
            insts = bb.instructions  # live list
            i = 0
            while i < len(insts):
                inst = insts[i]
                si = getattr(inst, "sync_info", None)
                if si is not None and len(si.on_wait) > 1:
                    extra, last = list(si.on_wait[:-1]), si.on_wait[-1]
                    for w in extra:
                        nop = mybir.InstNoOp(
                            name=nc.get_next_instruction_name(),
                            engine=inst.engine,
                            sync_info=mybir.SyncInfo(on_wait=[w], on_update=[]),
                            bass_nofuse=True,
                        )
                        insts.insert(i, nop)
                        i += 1
                    inst.sync_info = mybir.SyncInfo(
                        on_wait=[last], on_update=list(si.on_update)
                    )
                i += 1


def _build_program(bt, bg, w2, b2):
    import concourse.bass as bass
    import concourse.mybir as mybir
    from concourse.tile import TileContext

    nc = bass.Bass()
    x_d = nc.declare_dram_parameter("x", [ROWS, C], mybir.dt.float32, isOutput=False)
    g_d = nc.declare_dram_parameter("g", [ROWS, C], mybir.dt.float32, isOutput=False)
    w_d = nc.declare_dram_parameter("wcat", [P, CAT], mybir.dt.float32, isOutput=False)
    o_d = nc.declare_dram_parameter("out", [ROWS, C], mybir.dt.float32, isOutput=True)

    # Partition p owns pixel rows [p*RPP, (p+1)*RPP); block j covers slots
    # r in [j*TB, (j+1)*TB).  Per partition each block is TB*1KB contiguous
    # in HBM on both src and dst -> large DMA descriptors.
    x_r = x_d[:].rearrange("(p j r) c -> j p r c", p=P, j=NBLK, r=TB)
    g_r = g_d[:].rearrange("(p j r) c -> j p r c", p=P, j=NBLK, r=TB)
    o_r = o_d[:].rearrange("(p j r) c -> j p r c", p=P, j=NBLK, r=TB)

    f32 = mybir.dt.float32
    with TileContext(nc) as tc:
        with (
            tc.tile_pool(name="wp", bufs=1) as wp,
            tc.tile_pool(name="io", bufs=3) as io,
            tc.tile_pool(name="op", bufs=3) as op,
            tc.tile_pool(name="sc", bufs=2) as sc,
            tc.tile_pool(name="sm", bufs=4) as sm,
        ):
            wcat = wp.tile([P, 2, C], f32)   # [:,0,:]=wt  [:,1,:]=wg
            nc.sync.dma_start(wcat[:], w_d[:])
            b2t = wp.tile([P, 1], f32)
            nc.vector.memset(b2t[:], float(b2))
            bias_t = wp.tile([P, 1], f32)
            nc.vector.memset(bias_t[:], float(bt + bg))

            for j in range(NBLK):
                xg = io.tile([P, 2, TB, C], f32)   # [:,0]=x block, [:,1]=g block
                nc.sync.dma_start(xg[:, 0], x_r[j])
                nc.sync.dma_start(xg[:, 1], g_r[j])
                ob = op.tile([P, TB, C], f32)
                s_blk = sm.tile([P, TB], f32)
                for r in range(TB):
                    dump = sc.tile([P, 2, C], f32)
                    # dump = [x_row*wt, g_row*wg]; accum = full 512-dot = t+g
                    nc.vector.scalar_tensor_tensor(
                        out=dump[:],
                        in0=xg[:, :, r, :],
                        scalar=0.0,
                        in1=wcat[:],
                        op0=mybir.AluOpType.bypass,
                        op1=mybir.AluOpType.mult,
                        accum_out=s_blk[:, r : r + 1],
                    )
                xs_blk = sm.tile([P, TB], f32)
                nc.scalar.activation(
                    xs_blk[:], s_blk[:], mybir.ActivationFunctionType.Relu,
                    bias=bias_t[:],
                )
                xsig_blk = sm.tile([P, TB], f32)
                nc.scalar.activation(
                    xsig_blk[:], xs_blk[:], mybir.ActivationFunctionType.Sigmoid,
                    bias=b2t[:], scale=float(w2),
                )
                for r in range(TB):
                    nc.scalar.mul(
                        ob[:, r, :], xg[:, 0, r, :], xsig_blk[:, r : r + 1]
                    )
                nc.sync.dma_start(o_r[j], ob[:])
    _split_multi_waits(nc)
    return nc


def kernel(**inputs):
    _apply_compat_patches()
    from concourse.bass_utils import run_bass_kernel_spmd

    x = np.ascontiguousarray(inputs["input_"], dtype=np.float32)
    g = np.ascontiguousarray(inputs["gating_signal"], dtype=np.float32)
    wt = np.asarray(inputs["wt"], dtype=np.float32)
    wg = np.asarray(inputs["wg"], dtype=np.float32)
    bt = float(np.asarray(inputs["bt"]))
    bg = float(np.asarray(inputs["bg"]))
    w2 = float(np.asarray(inputs["w2"]))
    b2 = float(np.asarray(inputs["b2"]))

    nc = _build_program(bt, bg, w2, b2)

    wcat = np.ascontiguousarray(
        np.tile(np.concatenate([wt, wg])[None, :], (P, 1)).astype(np.float32)
    )
    xs = x.reshape(NCORES, ROWS, C)
    gs = g.reshape(NCORES, ROWS, C)
    in_maps = [{"x": xs[i], "g": gs[i], "wcat": wcat} for i in range(NCORES)]
    res = run_bass_kernel_spmd(nc, in_maps, list(range(NCORES)))
    out = np.stack([res.results[i]["out"] for i in range(NCORES)], axis=0)
    return out.reshape(B, H, W, C)


# revision 10
# speedup vs baseline: 1.2350x; 1.2144x over previous
"""AttentionGate kernel for Trainium2 (8 NeuronCores, pure data parallel).

Reference computation (per pixel p, channels c):
    t[p] = sum_c input_[p,c]*wt[c] + bt
    g[p] = sum_c gating [p,c]*wg[c] + bg
    x[p] = sigmoid(w2 * relu(t[p]+g[p]) + b2)
    out[p,c] = input_[p,c] * x[p]

Layout strategy: keep the natural [pixel, channel] layout (channel innermost,
contiguous in HBM).  Per 128-pixel tile we load input and gating channels into
one [128, 512] SBUF tile (concatenated along free dim); one fused custom-DVE
TENSOR_TENSOR_REDUCE computes the elementwise product with the concatenated
weights AND its free-dim sum (the dot product) in a single instruction.
ScalarE does relu + sigmoid on the [128,1] result; DVE tensor_scalar
broadcasts the per-pixel gate over the 256 input channels.

Sharding: batch dim 16 -> 2 batches per core, weights replicated.
"""

import sys

import numpy as np

for _p in ("/opt/trn_rl_repo", "/opt/trn_rl_repo/concourse"):
    if _p not in sys.path:
        sys.path.append(_p)

B, H, W, C = 16, 128, 128, 256
NCORES = 8
ROWS = (B // NCORES) * H * W          # pixels per core = 32768
P = 128                                # partitions
CAT = 2 * C                            # input || gating channels
RPP = ROWS // P                        # pixel rows owned per partition = 256
TB = 16                                # rows (slots) per block
NBLK = RPP // TB                       # 16 blocks

_PATCHED = False


def _apply_compat_patches():
    """Work around two ISA-encoding gaps in this container's neuronxcc walrus:

    1. EVENT_SEMAPHORE_RANGE_CLEAR (emitted by the TileContext teardown's
       sem_clear) fails codegen with "ISA wrong length".  Re-execution is
       safe without it (verified on HW), so skip the clear.
    2. The teardown drain carries one sem-wait per logical processor; this
       walrus rejects >1 sync-wait command on a NO_STRUCT ctrl instruction
       ("Too many sync wait commands").  Split the final clock wait into one
       NOP per processor instead.
    """
    global _PATCHED
    if _PATCHED:
        return
    _PATCHED = True

    import concourse.bass as bass
    import concourse.tile as tile_mod
    from bass_rust import ScopedClock, VectorClock
    from concourse.bass import SemaphoreHandle, compact_to_ranges

    def patched_clear(self, sems):
        if not sems:
            return
        sem_nums = [s.num if isinstance(s, SemaphoreHandle) else s for s in sems]
        for sem_range in compact_to_ranges(sem_nums):
            assert self._state.free_isdisjoint(sem_range)
            self.gpsimd.dma_reset(sem_range)
        self._state.prepend_free_semaphores(sem_nums)
        for poison_set in self._tile_sem_poison_stack:
            poison_set.update(sem_nums)

    bass.Bass.clear_and_free_semaphores = patched_clear

    def patched_drain_and_barrier(self, tick_clock, wait_clock):
        gc = tick_clock.global_clock
        for p in range(len(gc)):
            if gc[p] <= 0:
                continue
            vc = VectorClock()
            vc.require_at_least(p, gc[p])
            di = self.nc.sync.nop(nofuse=True)
            wait_clock.add_sem_waits(di.ins, ScopedClock({None: vc}))
        self.nc.all_engine_barrier()
        assert self.sems is not None
        popped = self.nc._tile_sem_poison_stack.pop()
        assert popped is self._sem_poison
        self.nc.clear_and_free_semaphores(list(self.sems.allocated().values()))
        self.nc.all_engine_barrier()

    tile_mod.TileContext._drain_and_barrier = patched_drain_and_barrier


def _split_multi_waits(nc):
    """This walrus build only encodes ONE sync-wait command per TPB
    instruction.  Hoist all-but-the-last wait of any instruction onto
    freshly inserted same-engine NoOps placed directly before it."""
    import concourse.mybir as mybir

    for f in nc.m.functions:
        for bb in f.blocks:
            insts = bb.instructions  # live list
            i = 0
            while i < len(insts):
                inst = insts[i]
                si = getattr(inst, "sync_info", None)
                if si is not None and len(si.on_wait) > 1:
                    extra, last = list(si.on_wait[:-1]), si.on_wait[-1]
                    for w in extra:
                        nop = mybir.InstNoOp(
                            name=nc.get_next_instruction_name(),
                            engine=inst.engine,
                            sync_info=mybir.SyncInfo(on_wait=[w], on_update=[]),
                            bass_nofuse=True,
                        )
                        insts.insert(i, nop)
                        i += 1
                    inst.sync_info = mybir.SyncInfo(
                        on_wait=[last], on_update=list(si.on_update)
                    )
                i += 1


def _build_program(bt, bg, w2, b2):
    import concourse.bass as bass
    import concourse.mybir as mybir
    from concourse.tile import TileContext

    nc = bass.Bass()
    x_d = nc.declare_dram_parameter("x", [ROWS, C], mybir.dt.float32, isOutput=False)
    g_d = nc.declare_dram_parameter("g", [ROWS, C], mybir.dt.float32, isOutput=False)
    w_d = nc.declare_dram_parameter("wcat", [P, CAT], mybir.dt.float32, isOutput=False)
    o_d = nc.declare_dram_parameter("out", [ROWS, C], mybir.dt.float32, isOutput=True)

    # Partition p owns pixel rows [p*RPP, (p+1)*RPP); block j covers slots
    # r in [j*TB, (j+1)*TB).  Per partition each block is TB*1KB contiguous
    # in HBM on both src and dst -> large DMA descriptors.
    x_r = x_d[:].rearrange("(p j r) c -> j p r c", p=P, j=NBLK, r=TB)
    g_r = g_d[:].rearrange("(p j r) c -> j p r c", p=P, j=NBLK, r=TB)
    o_r = o_d[:].rearrange("(p j r) c -> j p r c", p=P, j=NBLK, r=TB)

    f32 = mybir.dt.float32
    with TileContext(nc) as tc:
        with (
            tc.tile_pool(name="wp", bufs=1) as wp,
            tc.tile_pool(name="io", bufs=4) as io,
            tc.tile_pool(name="op", bufs=3) as op,
            tc.tile_pool(name="sc", bufs=2) as sc,
            tc.tile_pool(name="sm", bufs=4) as sm,
        ):
            wcat = wp.tile([P, 2, C], f32)   # [:,0,:]=wt  [:,1,:]=wg
            nc.sync.dma_start(wcat[:], w_d[:])
            b2t = wp.tile([P, 1], f32)
            nc.vector.memset(b2t[:], float(b2))
            bias_t = wp.tile([P, 1], f32)
            nc.vector.memset(bias_t[:], float(bt + bg))

            for j in range(NBLK):
                xg = io.tile([P, 2, TB, C], f32)   # [:,0]=x block, [:,1]=g block
                nc.sync.dma_start(xg[:, 0], x_r[j])
                nc.sync.dma_start(xg[:, 1], g_r[j])
                ob = op.tile([P, TB, C], f32)
                s_blk = sm.tile([P, TB], f32)
                for r in range(TB):
                    dump = sc.tile([P, 2, C], f32)
                    # dump = [x_row*wt, g_row*wg]; accum = full 512-dot = t+g
                    nc.vector.scalar_tensor_tensor(
                        out=dump[:],
                        in0=xg[:, :, r, :],
                        scalar=0.0,
                        in1=wcat[:],
                        op0=mybir.AluOpType.bypass,
                        op1=mybir.AluOpType.mult,
                        accum_out=s_blk[:, r : r + 1],
                    )
                xs_blk = sm.tile([P, TB], f32)
                nc.scalar.activation(
                    xs_blk[:], s_blk[:], mybir.ActivationFunctionType.Relu,
                    bias=bias_t[:],
                )
                xsig_blk = sm.tile([P, TB], f32)
                nc.scalar.activation(
                    xsig_blk[:], xs_blk[:], mybir.ActivationFunctionType.Sigmoid,
                    bias=b2t[:], scale=float(w2),
                )
                for r in range(TB):
                    nc.scalar.mul(
                        ob[:, r, :], xg[:, 0, r, :], xsig_blk[:, r : r + 1]
                    )
                # out-DMA from the ACT ring: ACT just produced ob, so this
                # issues with no waits and doesn't head-of-line block the
                # SP ring's input prefetch.
                nc.scalar.dma_start(o_r[j], ob[:])
    _split_multi_waits(nc)
    return nc


def kernel(**inputs):
    _apply_compat_patches()
    from concourse.bass_utils import run_bass_kernel_spmd

    x = np.ascontiguousarray(inputs["input_"], dtype=np.float32)
    g = np.ascontiguousarray(inputs["gating_signal"], dtype=np.float32)
    wt = np.asarray(inputs["wt"], dtype=np.float32)
    wg = np.asarray(inputs["wg"], dtype=np.float32)
    bt = float(np.asarray(inputs["bt"]))
    bg = float(np.asarray(inputs["bg"]))
    w2 = float(np.asarray(inputs["w2"]))
    b2 = float(np.asarray(inputs["b2"]))

    nc = _build_program(bt, bg, w2, b2)

    wcat = np.ascontiguousarray(
        np.tile(np.concatenate([wt, wg])[None, :], (P, 1)).astype(np.float32)
    )
    xs = x.reshape(NCORES, ROWS, C)
    gs = g.reshape(NCORES, ROWS, C)
    in_maps = [{"x": xs[i], "g": gs[i], "wcat": wcat} for i in range(NCORES)]
    res = run_bass_kernel_spmd(nc, in_maps, list(range(NCORES)))
    out = np.stack([res.results[i]["out"] for i in range(NCORES)], axis=0)
    return out.reshape(B, H, W, C)
